# revision 2
# baseline (speedup 1.0000x reference)
"""ComplexLayerScale Trainium2 kernel.

out[b,t,d] = (x_real + i*x_imag)[b,t,d] * (gamma_real + i*gamma_imag)[d]

Sharding: data-parallel over the batch dim (B=8 -> 8 NeuronCores), gamma
replicated. Per core: x shard [4096, 512] f32 per component; output stored
as interleaved (re, im) f32 pairs [4096, 1024] and viewed as complex64 on
the host (zero-copy).

Per-core schedule (memory-bound; ~32 MiB HBM traffic per core):
  - 8 supertiles of 512 rows, each seen as SBUF tiles [128, 2048] f32.
  - loads on nc.sync (HWDGE/SP ring), store on nc.scalar (HWDGE/ACT ring).
  - DVE: t1 = xr*gr, t2 = xi*gi, out_re = t1 - t2, out_im = t3 + t4
  - GPSIMD: t3 = xr*gi, t4 = xi*gr
  (4 DVE ops ~73us, 2 GPSIMD ops ~72us, DMA ~90us -> DMA-bound.)
"""

import numpy as np

# Problem shape (hardcoded per contract).
B, T, D = 8, 4096, 512
N_CORES = 8
P = 128           # SBUF partitions
R = 4             # rows of x per partition per supertile
W = R * D         # free elems per input tile (2048 f32 = 8KB/partition)
ROWS = P * R      # 512 rows per supertile
NT = T // ROWS    # 8 supertiles per core

_CACHE = {}


def _build_program():
    import concourse.bacc as bacc
    import concourse.bass as bass
    import concourse.mybir as mybir
    import concourse.tile as tile

    f32 = mybir.dt.float32
    nc = bacc.Bacc("TRN2", target_bir_lowering=False, debug=False,
                   num_devices=N_CORES)

    xr = nc.dram_tensor("xr", [T, D], f32, kind="ExternalInput")
    xi = nc.dram_tensor("xi", [T, D], f32, kind="ExternalInput")
    gr = nc.dram_tensor("gr", [D], f32, kind="ExternalInput")
    gi = nc.dram_tensor("gi", [D], f32, kind="ExternalInput")
    out = nc.dram_tensor("out", [T, 2 * D], f32, kind="ExternalOutput")

    with tile.TileContext(nc) as tc:
        with tc.tile_pool(name="gamma", bufs=1) as gpool, \
             tc.tile_pool(name="io", bufs=3) as iop, \
             tc.tile_pool(name="scratch", bufs=2) as sp, \
             tc.tile_pool(name="outp", bufs=2) as op:

            # Replicated gamma tiles [P, W]: each partition holds R copies
            # of gamma along the free dim (matches the x supertile layout).
            grt = gpool.tile([P, W], f32, tag="grt")
            git = gpool.tile([P, W], f32, tag="git")
            for g_dram, g_tile in ((gr, grt), (gi, git)):
                g_ap = g_dram[:]
                src = bass.AP(tensor=g_ap.tensor, offset=g_ap.offset,
                              ap=[[0, P], [0, R], [1, D]])
                dst = g_tile[:].rearrange("p (r d) -> p r d", r=R, d=D)
                nc.gpsimd.dma_start(out=dst, in_=src)

            for it in range(NT):
                r0 = it * ROWS
                xrt = iop.tile([P, W], f32, tag="xrt")
                xit = iop.tile([P, W], f32, tag="xit")
                nc.sync.dma_start(
                    out=xrt[:],
                    in_=xr[r0:r0 + ROWS].rearrange("(p r) d -> p (r d)",
                                                   p=P, r=R))
                nc.sync.dma_start(
                    out=xit[:],
                    in_=xi[r0:r0 + ROWS].rearrange("(p r) d -> p (r d)",
                                                   p=P, r=R))

                t1 = sp.tile([P, W], f32, tag="t1")
                t2 = sp.tile([P, W], f32, tag="t2")
                t3 = sp.tile([P, W], f32, tag="t3")
                t4 = sp.tile([P, W], f32, tag="t4")
                ot = op.tile([P, 2 * W], f32, tag="ot")

                nc.vector.tensor_mul(out=t1[:], in0=xrt[:], in1=grt[:])
                nc.vector.tensor_mul(out=t2[:], in0=xit[:], in1=git[:])
                nc.gpsimd.tensor_mul(out=t3[:], in0=xrt[:], in1=git[:])
                nc.gpsimd.tensor_mul(out=t4[:], in0=xit[:], in1=grt[:])

                # Interleaved (re, im) pairs: flat free index k -> offset 2k.
                pairs = ot[:].rearrange("p (w two) -> p w two", w=W, two=2)
                o_re = pairs[:, :, 0]
                o_im = pairs[:, :, 1]
                nc.vector.tensor_sub(out=o_re, in0=t1[:], in1=t2[:])
                nc.vector.tensor_add(out=o_im, in0=t3[:], in1=t4[:])

                nc.scalar.dma_start(
                    out=out[r0:r0 + ROWS].rearrange("(p r) d -> p (r d)",
                                                    p=P, r=R),
                    in_=ot[:])
    nc.compile()
    return nc


def _get_program():
    if "nc" not in _CACHE:
        _CACHE["nc"] = _build_program()
    return _CACHE["nc"]


def _in_maps(x_real, x_imag, gamma_real, gamma_imag):
    return [{
        "xr": np.ascontiguousarray(x_real[b], dtype=np.float32),
        "xi": np.ascontiguousarray(x_imag[b], dtype=np.float32),
        "gr": np.ascontiguousarray(gamma_real, dtype=np.float32),
        "gi": np.ascontiguousarray(gamma_imag, dtype=np.float32),
    } for b in range(N_CORES)]


def kernel(x_real, x_imag, gamma_real, gamma_imag):
    from concourse.bass_utils import run_bass_kernel_spmd

    nc = _get_program()
    res = run_bass_kernel_spmd(
        nc, _in_maps(x_real, x_imag, gamma_real, gamma_imag),
        list(range(N_CORES)))
    shards = [res.results[c]["out"].view(np.complex64) for c in range(N_CORES)]
    return np.stack(shards, axis=0)


def run_traced(x_real, x_imag, gamma_real, gamma_imag, **kw):
    """Profiled run (for test.py): returns BassKernelResults with
    exec_time_ns populated from the NTFF profile."""
    from concourse.bass_utils import run_bass_kernel_spmd

    nc = _get_program()
    return run_bass_kernel_spmd(
        nc, _in_maps(x_real, x_imag, gamma_real, gamma_imag),
        list(range(N_CORES)), trace=True, **kw)


# revision 4
# speedup vs baseline: 1.0236x; 1.0236x over previous
"""ComplexLayerScale Trainium2 kernel.

out[b,t,d] = (x_real + i*x_imag)[b,t,d] * (gamma_real + i*gamma_imag)[d]

Sharding: data-parallel over the batch dim (B=8 -> 8 NeuronCores), gamma
replicated. Per core: x shard [4096, 512] f32 per component; output stored
as interleaved (re, im) f32 pairs [4096, 1024] and viewed as complex64 on
the host (zero-copy).

Formulation (all compute ops contiguous-output; stride-2 writes on the DVE
measured 2.8x slower than contiguous, so the interleave is instead baked
into two host-precomputed gamma vectors):
  G1 = tile_R(interleave(gr,  gi))      # [2*R*D] f32, built on host (O(D))
  G2 = tile_R(interleave(-gi, gr))
  A   = dup2(xr) * G1                   # dup2 = stride-0 doubled read
  B   = dup2(xi) * G2
  out = A + B                           # pairs (re, im) fall out contiguous
since out[2k] = xr*gr - xi*gi and out[2k+1] = xr*gi + xi*gr.

Per-core schedule: 8 supertiles of 512 rows ([128, 2048] f32 inputs,
[128, 4096] outputs). Loads on nc.sync, stores on nc.scalar (separate HWDGE
rings). Muls on DVE; the adds are split DVE/GPSIMD to balance engine time
under the ~90us/core HBM roofline (32 MiB traffic at ~358 GB/s).
"""

import numpy as np

# Problem shape (hardcoded per contract).
B, T, D = 8, 4096, 512
N_CORES = 8
P = 128           # SBUF partitions
R = 4             # rows of x per partition per supertile
W = R * D         # free elems per input tile (2048 f32 = 8KB/partition)
ROWS = P * R      # 512 rows per supertile
NT = T // ROWS    # 8 supertiles per core
K_GPSIMD_ADD = 6  # supertiles whose final add runs on GPSIMD (rest on DVE)

_CACHE = {}


def _build_program():
    import concourse.bacc as bacc
    import concourse.bass as bass
    import concourse.mybir as mybir
    import concourse.tile as tile

    f32 = mybir.dt.float32
    nc = bacc.Bacc("TRN2", target_bir_lowering=False, debug=False,
                   num_devices=N_CORES)

    xr = nc.dram_tensor("xr", [T, D], f32, kind="ExternalInput")
    xi = nc.dram_tensor("xi", [T, D], f32, kind="ExternalInput")
    g1 = nc.dram_tensor("g1", [2 * W], f32, kind="ExternalInput")
    g2 = nc.dram_tensor("g2", [2 * W], f32, kind="ExternalInput")
    out = nc.dram_tensor("out", [T, 2 * D], f32, kind="ExternalOutput")

    def dup2(ap):
        # [P, W] -> [P, W, 2] with a stride-0 innermost dim (each element
        # read twice, feeding the (re, im) output pair).
        return ap.unsqueeze(2).broadcast_to([P, W, 2])

    with tile.TileContext(nc) as tc:
        with tc.tile_pool(name="gamma", bufs=1) as gpool, \
             tc.tile_pool(name="io", bufs=3) as iop, \
             tc.tile_pool(name="ab", bufs=2) as abp:

            # Broadcast the host-built gamma vectors to all 128 partitions.
            g1t = gpool.tile([P, 2 * W], f32, tag="g1t")
            g2t = gpool.tile([P, 2 * W], f32, tag="g2t")
            for g_dram, g_tile in ((g1, g1t), (g2, g2t)):
                g_ap = g_dram[:]
                src = bass.AP(tensor=g_ap.tensor, offset=g_ap.offset,
                              ap=[[0, P], [1, 2 * W]])
                nc.gpsimd.dma_start(out=g_tile[:], in_=src)

            for it in range(NT):
                r0 = it * ROWS
                xrt = iop.tile([P, W], f32, tag="xrt")
                xit = iop.tile([P, W], f32, tag="xit")
                nc.sync.dma_start(
                    out=xrt[:],
                    in_=xr[r0:r0 + ROWS].rearrange("(p r) d -> p (r d)",
                                                   p=P, r=R))
                nc.sync.dma_start(
                    out=xit[:],
                    in_=xi[r0:r0 + ROWS].rearrange("(p r) d -> p (r d)",
                                                   p=P, r=R))

                a = abp.tile([P, 2 * W], f32, tag="a")
                b = abp.tile([P, 2 * W], f32, tag="b")
                a3 = a[:].rearrange("p (w two) -> p w two", w=W, two=2)
                b3 = b[:].rearrange("p (w two) -> p w two", w=W, two=2)
                g13 = g1t[:].rearrange("p (w two) -> p w two", w=W, two=2)
                g23 = g2t[:].rearrange("p (w two) -> p w two", w=W, two=2)

                nc.vector.tensor_mul(out=a3, in0=dup2(xrt[:]), in1=g13)
                nc.vector.tensor_mul(out=b3, in0=dup2(xit[:]), in1=g23)
                # Final add, in place into `a` (the store reads `a`).
                eng = nc.gpsimd if it < K_GPSIMD_ADD else nc.vector
                eng.tensor_add(out=a[:], in0=a[:], in1=b[:])

                nc.scalar.dma_start(
                    out=out[r0:r0 + ROWS].rearrange("(p r) d -> p (r d)",
                                                    p=P, r=R),
                    in_=a[:])
    nc.compile()
    return nc


def _get_program():
    if "nc" not in _CACHE:
        _CACHE["nc"] = _build_program()
    return _CACHE["nc"]


def _gamma_vectors(gamma_real, gamma_imag):
    gr = np.asarray(gamma_real, dtype=np.float32)
    gi = np.asarray(gamma_imag, dtype=np.float32)
    g1 = np.tile(np.stack([gr, gi], axis=-1).ravel(), R)     # [2*R*D]
    g2 = np.tile(np.stack([-gi, gr], axis=-1).ravel(), R)
    return np.ascontiguousarray(g1), np.ascontiguousarray(g2)


def _in_maps(x_real, x_imag, gamma_real, gamma_imag):
    g1, g2 = _gamma_vectors(gamma_real, gamma_imag)
    return [{
        "xr": np.ascontiguousarray(x_real[b], dtype=np.float32),
        "xi": np.ascontiguousarray(x_imag[b], dtype=np.float32),
        "g1": g1,
        "g2": g2,
    } for b in range(N_CORES)]


def kernel(x_real, x_imag, gamma_real, gamma_imag):
    from concourse.bass_utils import run_bass_kernel_spmd

    nc = _get_program()
    res = run_bass_kernel_spmd(
        nc, _in_maps(x_real, x_imag, gamma_real, gamma_imag),
        list(range(N_CORES)))
    shards = [res.results[c]["out"].view(np.complex64) for c in range(N_CORES)]
    return np.stack(shards, axis=0)


def run_traced(x_real, x_imag, gamma_real, gamma_imag, **kw):
    """Profiled run (for test.py): returns BassKernelResults with
    exec_time_ns populated from the NTFF profile."""
    from concourse.bass_utils import run_bass_kernel_spmd

    nc = _get_program()
    return run_bass_kernel_spmd(
        nc, _in_maps(x_real, x_imag, gamma_real, gamma_imag),
        list(range(N_CORES)), trace=True, **kw)


# revision 7
# speedup vs baseline: 1.3056x; 1.2754x over previous
"""ComplexLayerScale Trainium2 kernel.

out[b,t,d] = (x_real + i*x_imag)[b,t,d] * (gamma_real + i*gamma_imag)[d]

Sharding: data-parallel over the batch dim (B=8 -> 8 NeuronCores), gamma
replicated. Per core: x shard [4096, 512] f32 per component; output stored
as interleaved (re, im) f32 pairs [4096, 1024] and viewed as complex64 on
the host (zero-copy).

Formulation (all compute ops contiguous-output; stride-2 writes on the DVE
measured 2.8x slower than contiguous, so the interleave is instead baked
into two host-precomputed gamma vectors):
  G1 = tile_R(interleave(gr,  gi))      # [2*R*D] f32, built on host (O(D))
  G2 = tile_R(interleave(-gi, gr))
  A   = dup2(xr) * G1                   # dup2 = stride-0 doubled read
  B   = dup2(xi) * G2
  out = A + B                           # pairs (re, im) fall out contiguous
since out[2k] = xr*gr - xi*gi and out[2k+1] = xr*gi + xi*gr.

Per-core schedule: 8 supertiles of 512 rows ([128, 2048] f32 inputs,
[128, 4096] outputs). Loads on nc.sync, stores on nc.scalar (separate HWDGE
rings). Muls on DVE; the adds are split DVE/GPSIMD to balance engine time
under the ~90us/core HBM roofline (32 MiB traffic at ~358 GB/s).
"""

import numpy as np

# Problem shape (hardcoded per contract).
B, T, D = 8, 4096, 512
N_CORES = 8
P = 128           # SBUF partitions
R = 4             # rows of x per partition per supertile
W = R * D         # free elems per input tile (2048 f32 = 8KB/partition)
ROWS = P * R      # 512 rows per supertile
NT = T // ROWS    # 8 supertiles per core

_CACHE = {}


def _build_program():
    import concourse.bacc as bacc
    import concourse.bass as bass
    import concourse.mybir as mybir
    import concourse.tile as tile

    f32 = mybir.dt.float32
    nc = bacc.Bacc("TRN2", target_bir_lowering=False, debug=False,
                   num_devices=N_CORES)

    xr = nc.dram_tensor("xr", [T, D], f32, kind="ExternalInput")
    xi = nc.dram_tensor("xi", [T, D], f32, kind="ExternalInput")
    g1 = nc.dram_tensor("g1", [2 * D], f32, kind="ExternalInput")
    g2 = nc.dram_tensor("g2", [2 * D], f32, kind="ExternalInput")
    out = nc.dram_tensor("out", [T, 2 * D], f32, kind="ExternalOutput")

    def dup2(ap):
        # [P, W] -> [P, R, D, 2]: each x element read twice (stride-0
        # innermost), feeding its (re, im) output pair.
        return (ap.rearrange("p (r d) -> p r d", r=R, d=D)
                .unsqueeze(3).broadcast_to([P, R, D, 2]))

    with tile.TileContext(nc) as tc:
        with tc.tile_pool(name="gamma", bufs=1) as gpool, \
             tc.tile_pool(name="io", bufs=3) as iop, \
             tc.tile_pool(name="ab", bufs=2) as abp:

            # Broadcast the host-built interleaved gamma vectors [2*D] to all
            # 128 partitions (pair-width only; the R-repeat is a stride-0 dim
            # in the compute-op APs).
            g1t = gpool.tile([P, 2 * D], f32, tag="g1t")
            g2t = gpool.tile([P, 2 * D], f32, tag="g2t")
            for g_dram, g_tile in ((g1, g1t), (g2, g2t)):
                g_ap = g_dram[:]
                src = bass.AP(tensor=g_ap.tensor, offset=g_ap.offset,
                              ap=[[0, P], [1, 2 * D]])
                nc.gpsimd.dma_start(out=g_tile[:], in_=src)

            def grep4(g_tile):
                # [P, 2D] -> [P, R, D, 2] with stride-0 R dim.
                return (g_tile[:].rearrange("p (d two) -> p d two", d=D, two=2)
                        .unsqueeze(1).broadcast_to([P, R, D, 2]))

            g1v = grep4(g1t)
            g2v = grep4(g2t)

            for it in range(NT):
                r0 = it * ROWS
                xrt = iop.tile([P, W], f32, tag="xrt")
                xit = iop.tile([P, W], f32, tag="xit")
                nc.sync.dma_start(
                    out=xrt[:],
                    in_=xr[r0:r0 + ROWS].rearrange("(p r) d -> p (r d)",
                                                   p=P, r=R))
                nc.sync.dma_start(
                    out=xit[:],
                    in_=xi[r0:r0 + ROWS].rearrange("(p r) d -> p (r d)",
                                                   p=P, r=R))

                a = abp.tile([P, 2 * W], f32, tag="a")
                b = abp.tile([P, 2 * W], f32, tag="b")
                a4 = a[:].rearrange("p (r d two) -> p r d two",
                                    r=R, d=D, two=2)
                b4 = b[:].rearrange("p (r d two) -> p r d two",
                                    r=R, d=D, two=2)

                nc.vector.tensor_mul(out=a4, in0=dup2(xrt[:]), in1=g1v)
                nc.vector.tensor_mul(out=b4, in0=dup2(xit[:]), in1=g2v)
                # Final add, in place into `a` (the store reads `a`).
                nc.vector.tensor_add(out=a[:], in0=a[:], in1=b[:])

                nc.scalar.dma_start(
                    out=out[r0:r0 + ROWS].rearrange("(p r) d -> p (r d)",
                                                    p=P, r=R),
                    in_=a[:])
    nc.compile()
    return nc


def _get_program():
    if "nc" not in _CACHE:
        _CACHE["nc"] = _build_program()
    return _CACHE["nc"]


def _gamma_vectors(gamma_real, gamma_imag):
    gr = np.asarray(gamma_real, dtype=np.float32)
    gi = np.asarray(gamma_imag, dtype=np.float32)
    g1 = np.stack([gr, gi], axis=-1).ravel()                 # [2*D]
    g2 = np.stack([-gi, gr], axis=-1).ravel()
    return np.ascontiguousarray(g1), np.ascontiguousarray(g2)


def _in_maps(x_real, x_imag, gamma_real, gamma_imag):
    g1, g2 = _gamma_vectors(gamma_real, gamma_imag)
    return [{
        "xr": np.ascontiguousarray(x_real[b], dtype=np.float32),
        "xi": np.ascontiguousarray(x_imag[b], dtype=np.float32),
        "g1": g1,
        "g2": g2,
    } for b in range(N_CORES)]


def kernel(x_real, x_imag, gamma_real, gamma_imag):
    from concourse.bass_utils import run_bass_kernel_spmd

    nc = _get_program()
    res = run_bass_kernel_spmd(
        nc, _in_maps(x_real, x_imag, gamma_real, gamma_imag),
        list(range(N_CORES)))
    shards = [res.results[c]["out"].view(np.complex64) for c in range(N_CORES)]
    return np.stack(shards, axis=0)


def run_traced(x_real, x_imag, gamma_real, gamma_imag, **kw):
    """Profiled run (for test.py): returns BassKernelResults with
    exec_time_ns populated from the NTFF profile."""
    from concourse.bass_utils import run_bass_kernel_spmd

    nc = _get_program()
    return run_bass_kernel_spmd(
        nc, _in_maps(x_real, x_imag, gamma_real, gamma_imag),
        list(range(N_CORES)), trace=True, **kw)


# revision 11
# speedup vs baseline: 1.3081x; 1.0020x over previous
"""ComplexLayerScale Trainium2 kernel.

out[b,t,d] = (x_real + i*x_imag)[b,t,d] * (gamma_real + i*gamma_imag)[d]

Sharding: data-parallel over the batch dim (B=8 -> 8 NeuronCores), gamma
replicated. Per core: x shard [4096, 512] f32 per component; output stored
as interleaved (re, im) f32 pairs [4096, 1024] and viewed as complex64 on
the host (zero-copy).

Formulation (all compute ops contiguous-output; stride-2 writes on the DVE
measured 2.8x slower than contiguous, so the interleave is instead baked
into two host-precomputed gamma vectors):
  G1 = tile_R(interleave(gr,  gi))      # [2*R*D] f32, built on host (O(D))
  G2 = tile_R(interleave(-gi, gr))
  A   = dup2(xr) * G1                   # dup2 = stride-0 doubled read
  B   = dup2(xi) * G2
  out = A + B                           # pairs (re, im) fall out contiguous
since out[2k] = xr*gr - xi*gi and out[2k+1] = xr*gi + xi*gr.

Per-core schedule: 8 supertiles of 512 rows ([128, 2048] f32 inputs,
[128, 4096] outputs). Loads on nc.sync, stores on nc.scalar (separate HWDGE
rings). Muls on DVE; the adds are split DVE/GPSIMD to balance engine time
under the ~90us/core HBM roofline (32 MiB traffic at ~358 GB/s).
"""

import numpy as np

# Problem shape (hardcoded per contract).
B, T, D = 8, 4096, 512
N_CORES = 8
P = 128           # SBUF partitions
R = 4             # rows of x per partition per supertile
W = R * D         # free elems per input tile (2048 f32 = 8KB/partition)
ROWS = P * R      # 512 rows per supertile
NT = T // ROWS    # 8 supertiles per core

_CACHE = {}


def _build_program():
    import concourse.bacc as bacc
    import concourse.bass as bass
    import concourse.mybir as mybir
    import concourse.tile as tile

    f32 = mybir.dt.float32
    nc = bacc.Bacc("TRN2", target_bir_lowering=False, debug=False,
                   num_devices=N_CORES)

    xr = nc.dram_tensor("xr", [T, D], f32, kind="ExternalInput")
    xi = nc.dram_tensor("xi", [T, D], f32, kind="ExternalInput")
    g1 = nc.dram_tensor("g1", [P, 2 * D], f32, kind="ExternalInput")
    g2 = nc.dram_tensor("g2", [P, 2 * D], f32, kind="ExternalInput")
    out = nc.dram_tensor("out", [T, 2 * D], f32, kind="ExternalOutput")

    def dup2(ap):
        # [P, W] -> [P, R, D, 2]: each x element read twice (stride-0
        # innermost), feeding its (re, im) output pair.
        return (ap.rearrange("p (r d) -> p r d", r=R, d=D)
                .unsqueeze(3).broadcast_to([P, R, D, 2]))

    with tile.TileContext(nc) as tc:
        with tc.tile_pool(name="gamma", bufs=1) as gpool, \
             tc.tile_pool(name="io", bufs=3) as iop, \
             tc.tile_pool(name="ab", bufs=2) as abp:

            # Host-replicated interleaved gamma [P, 2*D]: plain contiguous
            # loads on the (initially idle) scalar HWDGE ring, so gamma
            # arrives with the first x tiles. The R-repeat is a stride-0 dim
            # in the compute-op APs.
            g1t = gpool.tile([P, 2 * D], f32, tag="g1t")
            g2t = gpool.tile([P, 2 * D], f32, tag="g2t")
            nc.scalar.dma_start(out=g1t[:], in_=g1[:])
            nc.scalar.dma_start(out=g2t[:], in_=g2[:])

            def grep4(g_tile):
                # [P, 2D] -> [P, R, D, 2] with stride-0 R dim.
                return (g_tile[:].rearrange("p (d two) -> p d two", d=D, two=2)
                        .unsqueeze(1).broadcast_to([P, R, D, 2]))

            g1v = grep4(g1t)
            g2v = grep4(g2t)

            for it in range(NT):
                r0 = it * ROWS
                xrt = iop.tile([P, W], f32, tag="xrt")
                xit = iop.tile([P, W], f32, tag="xit")
                nc.sync.dma_start(
                    out=xrt[:],
                    in_=xr[r0:r0 + ROWS].rearrange("(p r) d -> p (r d)",
                                                   p=P, r=R))
                nc.sync.dma_start(
                    out=xit[:],
                    in_=xi[r0:r0 + ROWS].rearrange("(p r) d -> p (r d)",
                                                   p=P, r=R))

                a = abp.tile([P, 2 * W], f32, tag="a")
                b = abp.tile([P, 2 * W], f32, tag="b")
                a4 = a[:].rearrange("p (r d two) -> p r d two",
                                    r=R, d=D, two=2)
                b4 = b[:].rearrange("p (r d two) -> p r d two",
                                    r=R, d=D, two=2)

                nc.vector.tensor_mul(out=a4, in0=dup2(xrt[:]), in1=g1v)
                nc.vector.tensor_mul(out=b4, in0=dup2(xit[:]), in1=g2v)
                # Final add, in place into `a` (the store reads `a`). The
                # last supertile is split in half so the kernel's final
                # store is 1 MiB instead of 2 MiB (shorter drain tail).
                out_view = out[r0:r0 + ROWS].rearrange("(p r) d -> p (r d)",
                                                       p=P, r=R)
                n_chunks = 2 if it == NT - 1 else 1
                cw = 2 * W // n_chunks
                for c in range(n_chunks):
                    sl = slice(c * cw, (c + 1) * cw)
                    nc.vector.tensor_add(out=a[:, sl], in0=a[:, sl],
                                         in1=b[:, sl])
                    nc.scalar.dma_start(out=out_view[:, sl], in_=a[:, sl])
    nc.compile()
    return nc


def _get_program():
    if "nc" not in _CACHE:
        _CACHE["nc"] = _build_program()
    return _CACHE["nc"]


def _gamma_vectors(gamma_real, gamma_imag):
    gr = np.asarray(gamma_real, dtype=np.float32)
    gi = np.asarray(gamma_imag, dtype=np.float32)
    g1 = np.stack([gr, gi], axis=-1).ravel()                 # [2*D]
    g2 = np.stack([-gi, gr], axis=-1).ravel()
    g1 = np.ascontiguousarray(np.broadcast_to(g1, (P, 2 * D)))
    g2 = np.ascontiguousarray(np.broadcast_to(g2, (P, 2 * D)))
    return g1, g2


def _in_maps(x_real, x_imag, gamma_real, gamma_imag):
    g1, g2 = _gamma_vectors(gamma_real, gamma_imag)
    return [{
        "xr": np.ascontiguousarray(x_real[b], dtype=np.float32),
        "xi": np.ascontiguousarray(x_imag[b], dtype=np.float32),
        "g1": g1,
        "g2": g2,
    } for b in range(N_CORES)]


def kernel(x_real, x_imag, gamma_real, gamma_imag):
    from concourse.bass_utils import run_bass_kernel_spmd

    nc = _get_program()
    res = run_bass_kernel_spmd(
        nc, _in_maps(x_real, x_imag, gamma_real, gamma_imag),
        list(range(N_CORES)))
    shards = [res.results[c]["out"].view(np.complex64) for c in range(N_CORES)]
    return np.stack(shards, axis=0)


def run_traced(x_real, x_imag, gamma_real, gamma_imag, **kw):
    """Profiled run (for test.py): returns BassKernelResults with
    exec_time_ns populated from the NTFF profile."""
    from concourse.bass_utils import run_bass_kernel_spmd

    nc = _get_program()
    return run_bass_kernel_spmd(
        nc, _in_maps(x_real, x_imag, gamma_real, gamma_imag),
        list(range(N_CORES)), trace=True, **kw)
